# revision 6
# baseline (speedup 1.0000x reference)
"""Trainium2 Bass kernel for nn_MHA_65429531787938.

MHA with a faithful-quirk softmax over dim=0 (the batch axis, B=2).
For B=2 the batch-softmax collapses to an elementwise sigmoid:
    attn0 = sigmoid((s0 - s1)/SCALE),  attn1 = 1 - attn0
and (1-A0) @ V1 = colsum(V1) - A0 @ V1, so a single attention matrix
serves both batches.

Sharding: tensor-parallel over the 16 heads -> 2 heads per core
(columns of w_q/w_k/w_v, rows of W_o). Each core consumes the full x
and produces a partial output (its heads' contribution to out = vals @ W_o,
scaled by 0.25 and stored fp16); the host sums the 8 partials and
multiplies by 4.

v3 changes vs v2 (153992 ns):
  - Single fused pipeline: x chunks stream j-major/batch-minor, so the
    batch-stacked q/k slices for an s-range complete after each chunk
    PAIR; the scores+sigmoids for q-chunk 0 (and half of q-chunk 1) are
    hoisted into phase 1, filling the projection phase's DMA-stall PE
    gaps and (crucially) moving ~26us of sigmoid work onto the ACT
    engine while it would otherwise idle.  Phase 2 keeps the A-tile
    pipeline one q-chunk ahead, so its steady state is PE-bound, not
    sigmoid-bound.
  - c1 = colsum(V1) moved off the PE: a DVE free-axis reduce over the
    vT strip, applied during the vals b1 copy as (-psum + c1) with a
    per-partition scalar operand.  Kills the 16 colsum matmuls and the
    per-chunk rank-1 correction matmuls.
  - Unified PSUM layout: one 3-buf pool of [128,1024] tiles (2 banks
    each) serves Q/K proj, V proj, scores, and out-proj; a 2-buf
    [128,512] pool serves V transposes + AV accumulators.  8 banks total.
  - out-proj psum copies alternate DVE/GpSimd (GpSimd was idle);
    weights load on the scalar/vector DMA queues in parallel with the
    x rings.
"""

import numpy as np

import concourse.bacc as bacc
import concourse.mybir as mybir
import concourse.tile as tile
from concourse import bass_utils
from concourse.alu_op_type import AluOpType
from concourse.masks import make_identity

B, S, D, H = 2, 2048, 1024, 16
HD = 64
SCALE = float(D) ** 0.5
NCORES = 8
HPC = H // NCORES            # heads per core = 2
MS = HPC * HD                # per-core slice width = 128
P = 128
NJ = S // 512                # chunk pairs / q-chunks = 4
NTILES = S // P              # k tiles = 16
DT16 = mybir.dt.float16
F32 = mybir.dt.float32
OSCALE = 0.25                # fp16 partial-output scale (host multiplies by 4)
SIG = mybir.ActivationFunctionType.Sigmoid


def build():
    nc = bacc.Bacc("TRN2", target_bir_lowering=False, debug=False)

    # x arrives pre-transposed+cast on host: [B, D, S] fp16
    xt_d = nc.dram_tensor("xt", [B, D, S], DT16, kind="ExternalInput").ap()
    wq_d = nc.dram_tensor("wq", [D, MS], DT16, kind="ExternalInput").ap()
    wk_d = nc.dram_tensor("wk", [D, MS], DT16, kind="ExternalInput").ap()
    wv_d = nc.dram_tensor("wv", [D, MS], DT16, kind="ExternalInput").ap()
    wo_d = nc.dram_tensor("wo", [MS, D], DT16, kind="ExternalInput").ap()
    out_d = nc.dram_tensor("out", [B, S, D], DT16, kind="ExternalOutput").ap()

    with tile.TileContext(nc) as tc:
        with tc.tile_pool(name="persist", bufs=1) as pp, \
             tc.tile_pool(name="xt", bufs=6) as pxt, \
             tc.tile_pool(name="at", bufs=32) as pat, \
             tc.tile_pool(name="ot", bufs=4) as pot, \
             tc.tile_pool(name="pd", bufs=3, space="PSUM") as ppd, \
             tc.tile_pool(name="pav", bufs=2, space="PSUM") as pav:

            ident16 = pp.tile([P, P], DT16, name="ident16")
            make_identity(nc, ident16[:])

            # weights on the scalar-engine DMA queue so they ramp in
            # parallel with the x chunks on the sync/gpsimd rings
            w_sb = {}
            for name, dram in (("wq", wq_d), ("wk", wk_d), ("wv", wv_d)):
                t = pp.tile([P, D // P, MS], DT16, name=f"{name}_sb")
                nc.scalar.dma_start(t[:], dram.rearrange("(t p) m -> p t m", p=P))
                w_sb[name] = t
            wo_sb = pp.tile([P, 2, 512], DT16)
            nc.scalar.dma_start(wo_sb[:], wo_d.rearrange("p (c n) -> p c n", c=2))

            # big persistent tensors
            qsb = pp.tile([P, HPC, S], DT16)     # [(b,hd), head, qpos], b1 negated
            ksb = pp.tile([P, HPC, S], DT16)     # [(b,hd), head, kpos]
            vt_sb = pp.tile([P, B, S], DT16)     # [(h,hd), batch, kpos]
            v_sb = pp.tile([P, S // P, HPC, B, HD], DT16)  # [k, ktile, h, b, hd]
            vals_sb = pp.tile([P, B, S], DT16)   # [(h,hd), batch, qpos]
            c1_sb = pp.tile([P, 1], F32)         # colsum(V1) per (h,hd)

            at_tiles = {}

            def emit_sc(qc, tp):
                # scores + sigmoid for q-chunk qc, k-tile pair (2tp, 2tp+1)
                for h in range(HPC):
                    pd = ppd.tile([P, 1024], F32, tag="pd", name="pd")
                    for u in range(2):
                        t = tp * 2 + u
                        nc.tensor.matmul(
                            pd[:, u * 512:(u + 1) * 512],
                            ksb[:, h, t * P:(t + 1) * P],
                            qsb[:, h, qc * 512:(qc + 1) * 512],
                            start=True, stop=True,
                        )
                    at = pat.tile([P, 1024], DT16, tag="at", name="at")
                    nc.scalar.activation(at[:], pd[:], SIG, scale=1.0 / SCALE)
                    at_tiles[(qc, tp, h)] = at

            def emit_av(qc, tp, pavs):
                for h in range(HPC):
                    at = at_tiles.pop((qc, tp, h))
                    for u in range(2):
                        t = tp * 2 + u
                        nc.tensor.matmul(
                            pavs[h][:],
                            v_sb[:, t, h, :, :].rearrange("p b d -> p (b d)"),
                            at[:, u * 512:(u + 1) * 512],
                            start=(t == 0), stop=(t == NTILES - 1),
                        )

            def emit_out_block(b, si, ceng, ring):
                # out[b, si*128:(si+1)*128, :] = OSCALE * vals^T @ W_o
                po = ppd.tile([P, 1024], F32, tag="pd", name="po")
                for nch in range(2):
                    nc.tensor.matmul(
                        po[:, nch * 512:(nch + 1) * 512],
                        vals_sb[:, b, si * P:(si + 1) * P],
                        wo_sb[:, nch, :],
                        start=True, stop=True,
                    )
                ot = pot.tile([P, 1024], DT16, tag="ot", name="ot")
                if ceng is nc.scalar:
                    nc.scalar.mul(ot[:], po[:], OSCALE)
                else:
                    ceng.tensor_scalar_mul(ot[:], po[:], OSCALE)
                ring.dma_start(out_d[b, si * P:(si + 1) * P, :], ot[:])

            # ---------------- phase 1: projections + hoisted scores ----------
            # PE warm-up: the first real matmul waits for the cold DMA
            # queues, long enough for the HAM clock gate to throttle PE.
            # A chain of dummy transposes (WAW-serialized) keeps the
            # activity window busy; the second batch reads the first x
            # chunk so it bridges right up to the first projection.
            wt = pav.tile([P, P], DT16, tag="av", name="wt")
            for _ in range(60):
                nc.tensor.transpose(wt[:], ident16[:], ident16[:])

            for j in range(NJ):
                for b in range(B):
                    c = 2 * j + b
                    xt = pxt.tile([P, D // P, 512], DT16, tag="xt")
                    src = xt_d[b, :, j * 512:(j + 1) * 512].rearrange(
                        "(t p) s -> p t s", p=P)
                    if c == 0:
                        # split the very first load across both rings so
                        # the pipeline fills faster
                        nc.sync.dma_start(xt[:, :4, :], src[:, :4, :])
                        nc.gpsimd.dma_start(xt[:, 4:, :], src[:, 4:, :])
                        for _ in range(30):
                            nc.tensor.transpose(wt[:], xt[:, 0, 0:P],
                                                ident16[:])
                    else:
                        eng = nc.sync if c % 2 == 0 else nc.gpsimd
                        eng.dma_start(xt[:], src)
                    # Q and K projections into halves of one psum tile
                    pqk = ppd.tile([P, 1024], F32, tag="pd", name="pqk")
                    for half, name in ((0, "wq"), (1, "wk")):
                        for t in range(D // P):
                            nc.tensor.matmul(
                                pqk[:, half * 512:(half + 1) * 512],
                                w_sb[name][:, t, :], xt[:, t, :],
                                start=(t == 0), stop=(t == D // P - 1),
                            )
                    for half, dest, neg in ((0, qsb, True), (1, ksb, False)):
                        for h in range(HPC):
                            dst = dest[b * HD:(b + 1) * HD, h,
                                       j * 512:(j + 1) * 512]
                            sc = -1.0 if (neg and b == 1) else 1.0
                            nc.vector.tensor_scalar_mul(
                                dst,
                                pqk[h * HD:(h + 1) * HD,
                                    half * 512:(half + 1) * 512],
                                sc)
                    # V projection (vT layout, natural sign)
                    pv = ppd.tile([P, 1024], F32, tag="pd", name="pv")
                    for t in range(D // P):
                        nc.tensor.matmul(
                            pv[:, :512], w_sb["wv"][:, t, :], xt[:, t, :],
                            start=(t == 0), stop=(t == D // P - 1),
                        )
                    nc.vector.tensor_scalar_mul(
                        vt_sb[:, b, j * 512:(j + 1) * 512], pv[:, :512], 1.0)
                    # V natural layout for the 4 k-tiles of this chunk
                    pvt = pav.tile([P, 4, P], DT16, tag="av", name="pvt")
                    for blk in range(4):
                        t = j * 4 + blk
                        nc.tensor.transpose(
                            pvt[:, blk, :], vt_sb[:, b, t * P:(t + 1) * P],
                            ident16[:],
                        )
                    # (GpSimd cannot read PSUM; this stays on DVE)
                    nc.vector.tensor_copy(
                        v_sb[:, j * 4:(j + 1) * 4, :, b, :],
                        pvt[:].rearrange("p t (h d) -> p t h d", h=HPC),
                    )
                # hoisted scores: qc0 fully; qc1's first half on pairs 2,3.
                # Fills DMA-stall PE gaps, and runs ~26us of sigmoids on
                # the otherwise-idle ACT engine.
                emit_sc(0, 2 * j)
                emit_sc(0, 2 * j + 1)
                if j >= 2:
                    emit_sc(1, 2 * (j - 2))
                    emit_sc(1, 2 * (j - 2) + 1)

            # colsum(V1) off the PE: free-axis reduce over the vT strip
            nc.vector.reduce_sum(c1_sb[:], vt_sb[:, 1, :],
                                 axis=mybir.AxisListType.X)

            # ------------- phase 2: AV + next-qc scores + out-proj ----------
            for qc in range(NJ):
                pavs = [pav.tile([P, 512], F32, tag="av", name=f"pav{h}")
                        for h in range(HPC)]
                for tp in range(8):
                    # keep the A pipeline one q-chunk ahead
                    if qc == 0:
                        if tp % 2 == 0:
                            emit_sc(1, 4 + tp // 2)
                    elif qc < NJ - 1:
                        emit_sc(qc + 1, tp)
                    emit_av(qc, tp, pavs)
                    if qc > 0:
                        b, sq = divmod(tp, 4)
                        ring = nc.sync if tp % 2 == 0 else nc.gpsimd
                        emit_out_block(b, (qc - 1) * 4 + sq, nc.vector, ring)
                # vals copies: b0 plain, b1 = -psum + c1 (fused bias)
                for h in range(HPC):
                    nc.vector.tensor_copy(
                        vals_sb[h * HD:(h + 1) * HD, 0,
                                qc * 512:(qc + 1) * 512],
                        pavs[h][0:HD, :],
                    )
                    nc.vector.tensor_scalar(
                        vals_sb[h * HD:(h + 1) * HD, 1,
                                qc * 512:(qc + 1) * 512],
                        pavs[h][HD:2 * HD, :],
                        -1.0, c1_sb[h * HD:(h + 1) * HD, :],
                        AluOpType.mult, AluOpType.add,
                    )
            # trailing out-proj blocks for the last q-chunk (ACT is idle
            # in the tail, so alternate the psum copies DVE/ACT)
            for i in range(8):
                sq, b = divmod(i, B)
                ceng = nc.vector if i % 2 == 0 else nc.scalar
                ring = nc.sync if i % 2 == 0 else nc.gpsimd
                emit_out_block(b, (NJ - 1) * 4 + sq, ceng, ring)

    nc.compile()
    return nc


_NC = None


def _get_nc():
    global _NC
    if _NC is None:
        _NC = build()
    return _NC


def kernel(x, w_q, w_k, w_v, W_o, _trace=False):
    x = np.asarray(x, dtype=np.float32)
    # host-side shard prep: transpose+cast x once, slice+cast weights per core
    x16t = np.ascontiguousarray(
        x.transpose(0, 2, 1).astype(np.float16))          # [B, D, S]
    w_q = np.asarray(w_q, dtype=np.float32)
    w_k = np.asarray(w_k, dtype=np.float32)
    w_v = np.asarray(w_v, dtype=np.float32)
    W_o = np.asarray(W_o, dtype=np.float32)

    nc = _get_nc()
    in_maps = []
    for i in range(NCORES):
        cs = slice(i * MS, (i + 1) * MS)
        in_maps.append({
            "xt": x16t,
            "wq": np.ascontiguousarray(w_q[:, cs].astype(np.float16)),
            "wk": np.ascontiguousarray(w_k[:, cs].astype(np.float16)),
            "wv": np.ascontiguousarray(w_v[:, cs].astype(np.float16)),
            "wo": np.ascontiguousarray(W_o[cs, :].astype(np.float16)),
        })
    try:
        res = bass_utils.run_bass_kernel_spmd(
            nc, in_maps, core_ids=list(range(NCORES)), trace=_trace
        )
    except Exception:
        # transient NRT exec failures have been observed to succeed on retry
        res = bass_utils.run_bass_kernel_spmd(
            nc, in_maps, core_ids=list(range(NCORES)), trace=_trace
        )
    out = res.results[0]["out"].astype(np.float32)
    for i in range(1, NCORES):
        out += res.results[i]["out"].astype(np.float32)
    out *= 1.0 / OSCALE
    if _trace:
        return out, res
    return out


# revision 10
# speedup vs baseline: 1.0017x; 1.0017x over previous
"""Trainium2 Bass kernel for nn_MHA_65429531787938.

MHA with a faithful-quirk softmax over dim=0 (the batch axis, B=2).
For B=2 the batch-softmax collapses to an elementwise sigmoid:
    attn0 = sigmoid((s0 - s1)/SCALE),  attn1 = 1 - attn0
and (1-A0) @ V1 = colsum(V1) - A0 @ V1, so a single attention matrix
serves both batches.

Sharding: tensor-parallel over the 16 heads -> 2 heads per core
(columns of w_q/w_k/w_v, rows of W_o). Each core consumes the full x
and produces a partial output (its heads' contribution to out = vals @ W_o,
scaled by 0.25 and stored fp16); the host sums the 8 partials and
multiplies by 4.

v3 changes vs v2 (153992 ns):
  - Single fused pipeline: x chunks stream j-major/batch-minor, so the
    batch-stacked q/k slices for an s-range complete after each chunk
    PAIR; the scores+sigmoids for q-chunk 0 (and half of q-chunk 1) are
    hoisted into phase 1, filling the projection phase's DMA-stall PE
    gaps and (crucially) moving ~26us of sigmoid work onto the ACT
    engine while it would otherwise idle.  Phase 2 keeps the A-tile
    pipeline one q-chunk ahead, so its steady state is PE-bound, not
    sigmoid-bound.
  - c1 = colsum(V1) moved off the PE: a DVE free-axis reduce over the
    vT strip, applied during the vals b1 copy as (-psum + c1) with a
    per-partition scalar operand.  Kills the 16 colsum matmuls and the
    per-chunk rank-1 correction matmuls.
  - Unified PSUM layout: one 3-buf pool of [128,1024] tiles (2 banks
    each) serves Q/K proj, V proj, scores, and out-proj; a 2-buf
    [128,512] pool serves V transposes + AV accumulators.  8 banks total.
  - out-proj psum copies alternate DVE/GpSimd (GpSimd was idle);
    weights load on the scalar/vector DMA queues in parallel with the
    x rings.
"""

import numpy as np

import concourse.bacc as bacc
import concourse.mybir as mybir
import concourse.tile as tile
from concourse import bass_utils
from concourse.alu_op_type import AluOpType
from concourse.masks import make_identity

B, S, D, H = 2, 2048, 1024, 16
HD = 64
SCALE = float(D) ** 0.5
NCORES = 8
HPC = H // NCORES            # heads per core = 2
MS = HPC * HD                # per-core slice width = 128
P = 128
NJ = S // 512                # chunk pairs / q-chunks = 4
NTILES = S // P              # k tiles = 16
DT16 = mybir.dt.float16
F32 = mybir.dt.float32
OSCALE = 0.25                # fp16 partial-output scale (host multiplies by 4)
SIG = mybir.ActivationFunctionType.Sigmoid


def build():
    nc = bacc.Bacc("TRN2", target_bir_lowering=False, debug=False)

    # x arrives pre-transposed+cast on host: [B, D, S] fp16
    xt_d = nc.dram_tensor("xt", [B, D, S], DT16, kind="ExternalInput").ap()
    wq_d = nc.dram_tensor("wq", [D, MS], DT16, kind="ExternalInput").ap()
    wk_d = nc.dram_tensor("wk", [D, MS], DT16, kind="ExternalInput").ap()
    wv_d = nc.dram_tensor("wv", [D, MS], DT16, kind="ExternalInput").ap()
    wo_d = nc.dram_tensor("wo", [MS, D], DT16, kind="ExternalInput").ap()
    out_d = nc.dram_tensor("out", [B, S, D], DT16, kind="ExternalOutput").ap()

    with tile.TileContext(nc) as tc:
        with tc.tile_pool(name="persist", bufs=1) as pp, \
             tc.tile_pool(name="xt", bufs=6) as pxt, \
             tc.tile_pool(name="at", bufs=32) as pat, \
             tc.tile_pool(name="ot", bufs=4) as pot, \
             tc.tile_pool(name="pd", bufs=3, space="PSUM") as ppd, \
             tc.tile_pool(name="pav", bufs=2, space="PSUM") as pav:

            ident16 = pp.tile([P, P], DT16, name="ident16")
            make_identity(nc, ident16[:])

            # weight loads split across the fast rings ahead of the x
            # chunks each ring carries: wq/wk lead the gpsimd ring (so
            # the first projections can start), wv queues on sync BEHIND
            # chunk 0's half (V proj runs after Q/K so it tolerates the
            # later arrival; this keeps c0 earliest).  wo is not needed
            # until the first out-proj (~half way in), so it rides the
            # slow scalar-engine queue.
            w_sb = {}
            for name, dram in (("wq", wq_d), ("wk", wk_d), ("wv", wv_d)):
                t = pp.tile([P, D // P, MS], DT16, name=f"{name}_sb")
                if name != "wv":
                    nc.gpsimd.dma_start(
                        t[:], dram.rearrange("(t p) m -> p t m", p=P))
                w_sb[name] = t
            wo_sb = pp.tile([P, 2, 512], DT16)

            # big persistent tensors
            qsb = pp.tile([P, HPC, S], DT16)     # [(b,hd), head, qpos], b1 negated
            ksb = pp.tile([P, HPC, S], DT16)     # [(b,hd), head, kpos]
            vt_sb = pp.tile([P, B, S], DT16)     # [(h,hd), batch, kpos]
            v_sb = pp.tile([P, S // P, HPC, B, HD], DT16)  # [k, ktile, h, b, hd]
            vals_sb = pp.tile([P, B, S], DT16)   # [(h,hd), batch, qpos]
            c1_sb = pp.tile([P, 1], F32)         # colsum(V1) per (h,hd)

            at_tiles = {}

            def emit_sc(qc, tp):
                # scores + sigmoid for q-chunk qc, k-tile pair (2tp, 2tp+1)
                for h in range(HPC):
                    pd = ppd.tile([P, 1024], F32, tag="pd", name="pd")
                    for u in range(2):
                        t = tp * 2 + u
                        nc.tensor.matmul(
                            pd[:, u * 512:(u + 1) * 512],
                            ksb[:, h, t * P:(t + 1) * P],
                            qsb[:, h, qc * 512:(qc + 1) * 512],
                            start=True, stop=True,
                        )
                    at = pat.tile([P, 1024], DT16, tag="at", name="at")
                    nc.scalar.activation(at[:], pd[:], SIG, scale=1.0 / SCALE)
                    at_tiles[(qc, tp, h)] = at

            def emit_av(qc, tp, pavs):
                for h in range(HPC):
                    at = at_tiles.pop((qc, tp, h))
                    for u in range(2):
                        t = tp * 2 + u
                        nc.tensor.matmul(
                            pavs[h][:],
                            v_sb[:, t, h, :, :].rearrange("p b d -> p (b d)"),
                            at[:, u * 512:(u + 1) * 512],
                            start=(t == 0), stop=(t == NTILES - 1),
                        )

            def emit_out_block(b, si, ceng, ring):
                # out[b, si*128:(si+1)*128, :] = OSCALE * vals^T @ W_o
                po = ppd.tile([P, 1024], F32, tag="pd", name="po")
                for nch in range(2):
                    nc.tensor.matmul(
                        po[:, nch * 512:(nch + 1) * 512],
                        vals_sb[:, b, si * P:(si + 1) * P],
                        wo_sb[:, nch, :],
                        start=True, stop=True,
                    )
                ot = pot.tile([P, 1024], DT16, tag="ot", name="ot")
                if ceng is nc.scalar:
                    nc.scalar.mul(ot[:], po[:], OSCALE)
                else:
                    ceng.tensor_scalar_mul(ot[:], po[:], OSCALE)
                ring.dma_start(out_d[b, si * P:(si + 1) * P, :], ot[:])

            # ---------------- phase 1: projections + hoisted scores ----------
            # PE warm-up: the first real matmul waits for the cold DMA
            # queues, long enough for the HAM clock gate to throttle PE.
            # A chain of dummy transposes (WAW-serialized) keeps the
            # activity window busy; the second batch reads the first x
            # chunk so it bridges right up to the first projection.
            wt = pav.tile([P, P], DT16, tag="av", name="wt")
            for _ in range(60):
                nc.tensor.transpose(wt[:], ident16[:], ident16[:])

            # Hoisted score blocks, one per slot so the pd ring never
            # queues two sigmoids back-to-back ahead of a projection:
            #   chunk (j,b1) slots: sc(0, 2j), sc(0, 2j+1)   (pair j done)
            #   chunk (j,b0) slots, j>=2: sc(1, 2(j-2)..)    (pair 1 done)
            # This runs qc0's (and half of qc1's) sigmoids on the
            # otherwise-idle ACT engine and fills DMA-stall PE gaps.
            def hoist_slots(j, b):
                if b == 1:
                    return [(0, 2 * j), (0, 2 * j + 1)]
                if j >= 2:
                    return [(1, 2 * (j - 2)), (1, 2 * (j - 2) + 1)]
                return [None, None]

            for j in range(NJ):
                for b in range(B):
                    c = 2 * j + b
                    slots = hoist_slots(j, b)
                    xt = pxt.tile([P, D // P, 512], DT16, tag="xt")
                    src = xt_d[b, :, j * 512:(j + 1) * 512].rearrange(
                        "(t p) s -> p t s", p=P)
                    if c == 0:
                        # split the very first load across both rings so
                        # the pipeline fills faster; wv/wo queue behind it
                        nc.sync.dma_start(xt[:, :4, :], src[:, :4, :])
                        nc.gpsimd.dma_start(xt[:, 4:, :], src[:, 4:, :])
                        nc.sync.dma_start(
                            w_sb["wv"][:],
                            wv_d.rearrange("(t p) m -> p t m", p=P))
                        nc.scalar.dma_start(
                            wo_sb[:], wo_d.rearrange("p (c n) -> p c n", c=2))
                        for _ in range(30):
                            nc.tensor.transpose(wt[:], xt[:, 0, 0:P],
                                                ident16[:])
                    else:
                        eng = nc.sync if c % 2 == 0 else nc.gpsimd
                        eng.dma_start(xt[:], src)
                    # Q and K projections into halves of one psum tile
                    pqk = ppd.tile([P, 1024], F32, tag="pd", name="pqk")
                    for half, name in ((0, "wq"), (1, "wk")):
                        for t in range(D // P):
                            nc.tensor.matmul(
                                pqk[:, half * 512:(half + 1) * 512],
                                w_sb[name][:, t, :], xt[:, t, :],
                                start=(t == 0), stop=(t == D // P - 1),
                            )
                    for half, dest, neg in ((0, qsb, True), (1, ksb, False)):
                        for h in range(HPC):
                            dst = dest[b * HD:(b + 1) * HD, h,
                                       j * 512:(j + 1) * 512]
                            sc = -1.0 if (neg and b == 1) else 1.0
                            nc.vector.tensor_scalar_mul(
                                dst,
                                pqk[h * HD:(h + 1) * HD,
                                    half * 512:(half + 1) * 512],
                                sc)
                    if slots[0] is not None:
                        emit_sc(*slots[0])
                    # V projection (vT layout, natural sign)
                    pv = ppd.tile([P, 1024], F32, tag="pd", name="pv")
                    for t in range(D // P):
                        nc.tensor.matmul(
                            pv[:, :512], w_sb["wv"][:, t, :], xt[:, t, :],
                            start=(t == 0), stop=(t == D // P - 1),
                        )
                    nc.vector.tensor_scalar_mul(
                        vt_sb[:, b, j * 512:(j + 1) * 512], pv[:, :512], 1.0)
                    # V natural layout for the 4 k-tiles of this chunk
                    pvt = pav.tile([P, 4, P], DT16, tag="av", name="pvt")
                    for blk in range(4):
                        t = j * 4 + blk
                        nc.tensor.transpose(
                            pvt[:, blk, :], vt_sb[:, b, t * P:(t + 1) * P],
                            ident16[:],
                        )
                    # (GpSimd cannot read PSUM; this stays on DVE)
                    nc.vector.tensor_copy(
                        v_sb[:, j * 4:(j + 1) * 4, :, b, :],
                        pvt[:].rearrange("p t (h d) -> p t h d", h=HPC),
                    )
                    if slots[1] is not None:
                        emit_sc(*slots[1])

            # colsum(V1) off the PE: free-axis reduce over the vT strip
            nc.vector.reduce_sum(c1_sb[:], vt_sb[:, 1, :],
                                 axis=mybir.AxisListType.X)

            # ------------- phase 2: AV + next-qc scores + out-proj ----------
            # Remaining score blocks spread so no iteration's sigmoid load
            # (1.1us each) exceeds its PE work; the A pipeline stays ~one
            # q-chunk ahead of the AV consumer throughout.
            sc_items = {
                0: [(1, 4), (1, 5), (1, 6), (1, 7), (2, 0), (2, 1)],
                1: [(2, 2), (2, 3), (2, 4), (2, 5), (2, 6), (2, 7)],
                2: [(3, 0), (3, 1), (3, 2), (3, 3), (3, 4), (3, 5)],
                3: [(3, 6), (3, 7)],
            }
            for qc in range(NJ):
                sched = {}
                n = len(sc_items[qc])
                for k, it in enumerate(sc_items[qc]):
                    sched.setdefault(k * 8 // n, []).append(it)
                pavs = [pav.tile([P, 512], F32, tag="av", name=f"pav{h}")
                        for h in range(HPC)]
                for tp in range(8):
                    for it in sched.get(tp, ()):
                        emit_sc(*it)
                    emit_av(qc, tp, pavs)
                    if qc > 0:
                        b, sq = divmod(tp, 4)
                        ring = nc.sync if tp % 2 == 0 else nc.gpsimd
                        emit_out_block(b, (qc - 1) * 4 + sq, nc.vector, ring)
                # vals copies: b0 plain, b1 = -psum + c1 (fused bias)
                for h in range(HPC):
                    nc.vector.tensor_copy(
                        vals_sb[h * HD:(h + 1) * HD, 0,
                                qc * 512:(qc + 1) * 512],
                        pavs[h][0:HD, :],
                    )
                    nc.vector.tensor_scalar(
                        vals_sb[h * HD:(h + 1) * HD, 1,
                                qc * 512:(qc + 1) * 512],
                        pavs[h][HD:2 * HD, :],
                        -1.0, c1_sb[h * HD:(h + 1) * HD, :],
                        AluOpType.mult, AluOpType.add,
                    )
            # trailing out-proj blocks for the last q-chunk (ACT is idle
            # in the tail, so alternate the psum copies DVE/ACT)
            for i in range(8):
                sq, b = divmod(i, B)
                ceng = nc.vector if i % 2 == 0 else nc.scalar
                ring = nc.sync if i % 2 == 0 else nc.gpsimd
                emit_out_block(b, (NJ - 1) * 4 + sq, ceng, ring)

    nc.compile()
    return nc


_NC = None


def _get_nc():
    global _NC
    if _NC is None:
        _NC = build()
    return _NC


def kernel(x, w_q, w_k, w_v, W_o, _trace=False):
    x = np.asarray(x, dtype=np.float32)
    # host-side shard prep: transpose+cast x once, slice+cast weights per core
    x16t = np.ascontiguousarray(
        x.transpose(0, 2, 1).astype(np.float16))          # [B, D, S]
    w_q = np.asarray(w_q, dtype=np.float32)
    w_k = np.asarray(w_k, dtype=np.float32)
    w_v = np.asarray(w_v, dtype=np.float32)
    W_o = np.asarray(W_o, dtype=np.float32)

    nc = _get_nc()
    in_maps = []
    for i in range(NCORES):
        cs = slice(i * MS, (i + 1) * MS)
        in_maps.append({
            "xt": x16t,
            "wq": np.ascontiguousarray(w_q[:, cs].astype(np.float16)),
            "wk": np.ascontiguousarray(w_k[:, cs].astype(np.float16)),
            "wv": np.ascontiguousarray(w_v[:, cs].astype(np.float16)),
            "wo": np.ascontiguousarray(W_o[cs, :].astype(np.float16)),
        })
    try:
        res = bass_utils.run_bass_kernel_spmd(
            nc, in_maps, core_ids=list(range(NCORES)), trace=_trace
        )
    except Exception:
        # transient NRT exec failures have been observed to succeed on retry
        res = bass_utils.run_bass_kernel_spmd(
            nc, in_maps, core_ids=list(range(NCORES)), trace=_trace
        )
    out = res.results[0]["out"].astype(np.float32)
    for i in range(1, NCORES):
        out += res.results[i]["out"].astype(np.float32)
    out *= 1.0 / OSCALE
    if _trace:
        return out, res
    return out
